# revision 16
# baseline (speedup 1.0000x reference)
"""Trainium2 Bass kernel for nn_Cross_Attention_Block_3624952397825.

Mathematical structure exploited: the reference takes ``out[:, -1, :]`` --
the attention output of the LAST query token. That token comes from the
zero row appended by ``jnp.pad`` AFTER the conv stack, so its query vector
is exactly zero, its attention scores are exactly zero, and softmax over
exact zeros is exactly uniform (1/4096).  Hence

    bins[b]  = mean_k V[b, k, :]          = (mean_k lidar[b, k, :]) @ wv
    out[b]   = MLP3(leaky_relu chain)(bins[b])

The conv block, Q/K projections, and softmax are structurally dead code
for ANY input values.  The kernel therefore reduces lidar over its 4096
points on-device, applies wv (pre-scaled by 1/4096 on the host) and the
3-layer MLP, all on 8 NeuronCores data-parallel over the batch (2
batches per core).

Implementation notes:
  * The point reduction runs on TensorE as ones^T @ tile matmuls in
    float32r (full-rate streaming; exact since the stationary operand is
    exactly 1.0), accumulating in PSUM across a batch's 16 [128, 512]
    slices.  VectorE only folds the final [1, 512] -> [1, 256].
  * MLP weights/biases/constants ship in one packed [128, 3463] tensor,
    split into 8 column-chunk DMAs issued BEFORE the lidar stream so
    weights are on-chip early; lidar DMAs go batch 0 first so batch 0's
    MLP overlaps batch 1's stream.
  * Activations flow transposed ([features, batch-column]) so biases are
    per-partition ScalarE activation biases; leaky_relu is
    (z*0.01) max z on VectorE.
"""

import numpy as np

B, NPTS, CH, DM = 16, 4096, 256, 1024
N_CORES = 8
BL = B // N_CORES            # batches per core
P = 128
TILE_F = 1024                # free dim of lidar tiles (4 pts x 256 ch)
N_TILES = NPTS * CH // (P * TILE_F)   # 8 tiles per batch
MM_F = 256                   # moving free dim per reduction matmul (one point)

# wpack free-dim layout
OFF_WVS = 0                  # 2 k-chunks x 1024
OFF_WO1 = 2048               # 8 k-chunks x 128
OFF_WO2 = 3072               # 128
OFF_WO3 = 3200               # 256
OFF_B = 3456                 # b1, b2, b3[:128], b3[128:]
OFF_ONE = 3460               # column of ones (fp32)
WPACK_F = 3463
N_WCHUNK = 8                 # weight DMA split

_CACHE = {}


def _build_program():
    import concourse.bacc as bacc
    import concourse.mybir as mybir
    from concourse.tile import TileContext

    f32 = mybir.dt.float32
    f32r = mybir.dt.float32r
    Alu = mybir.AluOpType
    Act = mybir.ActivationFunctionType

    nc = bacc.Bacc("TRN2")
    lidar = nc.dram_tensor("lidar", [BL, NPTS, CH], f32r, kind="ExternalInput")
    wpack = nc.dram_tensor("wpack", [P, WPACK_F], f32, kind="ExternalInput")
    ones_d = nc.dram_tensor("ones_d", [P, 1], f32r, kind="ExternalInput")
    out_rows = nc.dram_tensor("out_rows", [BL, CH], f32, kind="ExternalOutput")

    # [BL, 4096, 256] -> [(b t), 128, 1024]; per-partition rows are 4 KiB
    # contiguous in DRAM.
    lv = lidar[:, :, :].rearrange("b (t p q) c -> (b t) p (q c)", p=P, q=4)

    with TileContext(nc) as tc:
        with (
            tc.tile_pool(name="w", bufs=1) as wpool,
            tc.tile_pool(name="io", bufs=6) as iopool,
            tc.tile_pool(name="small", bufs=1) as spool,
            tc.tile_pool(name="ps", bufs=2, space="PSUM") as pspool,
            tc.tile_pool(name="mm", bufs=3, space="PSUM") as mmpool,
        ):
            # weights first, split across all DMA queues so they land early
            wp = wpool.tile([P, WPACK_F], f32, tag="wp")
            wc = (WPACK_F + N_WCHUNK - 1) // N_WCHUNK
            for j in range(N_WCHUNK):
                lo, hi = j * wc, min((j + 1) * wc, WPACK_F)
                nc.sync.dma_start(out=wp[:, lo:hi], in_=wpack[:, lo:hi])

            ones_t = spool.tile([P, 1], f32r, tag="ones_t")
            nc.sync.dma_start(out=ones_t[:, :], in_=ones_d[:, :])
            ones_r = ones_t[:, :]

            for b in range(BL):
                # ---- point reduction on TensorE (fp32r, exact for ones) ----
                sred = pspool.tile([1, MM_F], f32, tag="sred")
                nmm = N_TILES * (TILE_F // MM_F)
                i = 0
                for t in range(N_TILES):
                    tin = iopool.tile([P, TILE_F], f32r, tag="tin")
                    nc.sync.dma_start(out=tin[:, :], in_=lv[b * N_TILES + t, :, :])
                    for j in range(TILE_F // MM_F):
                        nc.tensor.matmul(sred[:, :], lhsT=ones_r,
                                         rhs=tin[:, j * MM_F:(j + 1) * MM_F],
                                         start=(i == 0), stop=(i == nmm - 1))
                        i += 1
                # with MM_F == CH every slice is one point's channels, so
                # sred is already the [1, 256] channel sums
                s_sb = spool.tile([1, CH], f32, tag=f"ssb{b}")
                nc.scalar.copy(out=s_sb[:, :], in_=sred[0:1, 0:CH])
                # transpose row [1, 256] -> 2 x [128, 1] via K=1 matmuls
                mt = []
                for k in range(2):
                    mtp = mmpool.tile([P, 1], f32, tag="mm")
                    nc.tensor.matmul(mtp[:, :], lhsT=s_sb[0:1, k * P:(k + 1) * P],
                                     rhs=wp[0:1, OFF_ONE:OFF_ONE + 1],
                                     start=True, stop=True)
                    mt_sb = spool.tile([P, 1], f32, tag=f"mt{b}{k}")
                    nc.scalar.copy(out=mt_sb[:, :], in_=mtp[:, :])
                    mt.append(mt_sb)

                # ---- v = sums @ (wv/4096): [1024, 1] transposed ----
                vS = spool.tile([P, 8], f32, tag=f"vS{b}")
                for o in range(8):
                    vps = mmpool.tile([P, 1], f32, tag="mm")
                    for k in range(2):
                        base = OFF_WVS + k * 1024 + o * P
                        nc.tensor.matmul(vps[:, :], lhsT=wp[:, base:base + P],
                                         rhs=mt[k][:, :], start=(k == 0), stop=(k == 1))
                    nc.scalar.copy(out=vS[:, o:o + 1], in_=vps[:, :])

                def leaky(zp, bias_col, tag):
                    z = spool.tile([P, 1], f32, tag=f"z{tag}")
                    nc.scalar.activation(z[:, :], zp[:, :], Act.Identity,
                                         bias=wp[:, bias_col:bias_col + 1], scale=1.0)
                    h = spool.tile([P, 1], f32, tag=f"h{tag}")
                    nc.vector.scalar_tensor_tensor(out=h[:, :], in0=z[:, :], scalar=0.01,
                                                   in1=z[:, :], op0=Alu.mult, op1=Alu.max)
                    return h

                h1p = mmpool.tile([P, 1], f32, tag="mm")
                for k in range(8):
                    nc.tensor.matmul(h1p[:, :],
                                     lhsT=wp[:, OFF_WO1 + k * P: OFF_WO1 + (k + 1) * P],
                                     rhs=vS[:, k:k + 1], start=(k == 0), stop=(k == 7))
                h1 = leaky(h1p, OFF_B + 0, f"1{b}")

                h2p = mmpool.tile([P, 1], f32, tag="mm")
                nc.tensor.matmul(h2p[:, :], lhsT=wp[:, OFF_WO2:OFF_WO2 + P],
                                 rhs=h1[:, :], start=True, stop=True)
                h2 = leaky(h2p, OFF_B + 1, f"2{b}")

                for o in range(2):
                    ops = mmpool.tile([P, 1], f32, tag="mm")
                    nc.tensor.matmul(ops[:, :],
                                     lhsT=wp[:, OFF_WO3 + o * P: OFF_WO3 + (o + 1) * P],
                                     rhs=h2[:, :], start=True, stop=True)
                    ofin = spool.tile([P, 1], f32, tag=f"ofin{b}{o}")
                    nc.scalar.activation(ofin[:, :], ops[:, :], Act.Identity,
                                         bias=wp[:, OFF_B + 2 + o:OFF_B + 3 + o], scale=1.0)
                    nc.sync.dma_start(out=out_rows[b:b + 1, o * P:(o + 1) * P],
                                      in_=ofin[:, :])

    nc.compile()
    return nc


def _pack_weights(inputs):
    wv = np.asarray(inputs["wv"], np.float32)
    wo1 = np.asarray(inputs["wo1"], np.float32)
    wo2 = np.asarray(inputs["wo2"], np.float32)
    wo3 = np.asarray(inputs["wo3"], np.float32)
    b1 = np.asarray(inputs["b1"], np.float32)
    b2 = np.asarray(inputs["b2"], np.float32)
    b3 = np.asarray(inputs["b3"], np.float32)

    wvs = wv * np.float32(1.0 / NPTS)        # fold the mean scale into wv
    wpack = np.zeros((P, WPACK_F), np.float32)
    wpack[:, OFF_WVS:OFF_WVS + 1024] = wvs[0:128, :]
    wpack[:, OFF_WVS + 1024:OFF_WVS + 2048] = wvs[128:256, :]
    for k in range(8):
        wpack[:, OFF_WO1 + k * P:OFF_WO1 + (k + 1) * P] = wo1[k * P:(k + 1) * P, :]
    wpack[:, OFF_WO2:OFF_WO2 + P] = wo2
    wpack[:, OFF_WO3:OFF_WO3 + CH] = wo3
    wpack[:, OFF_B + 0] = b1
    wpack[:, OFF_B + 1] = b2
    wpack[:, OFF_B + 2] = b3[0:128]
    wpack[:, OFF_B + 3] = b3[128:256]
    wpack[:, OFF_ONE] = 1.0
    return wpack


def kernel(**inputs):
    from concourse.bass_utils import run_bass_kernel_spmd

    if "nc" not in _CACHE:
        _CACHE["nc"] = _build_program()
    nc = _CACHE["nc"]

    lidar = np.ascontiguousarray(np.asarray(inputs["lidar"], dtype=np.float32))
    wpack = _pack_weights(inputs)

    ones_col = np.ones((P, 1), np.float32)
    in_maps = [
        {"lidar": lidar[i * BL:(i + 1) * BL], "wpack": wpack, "ones_d": ones_col}
        for i in range(N_CORES)
    ]
    res = run_bass_kernel_spmd(nc, in_maps, list(range(N_CORES)),
                               **_CACHE.get("run_kwargs", {}))
    _CACHE["last_results"] = res
    out = np.concatenate([res.results[i]["out_rows"] for i in range(N_CORES)], axis=0)
    return np.ascontiguousarray(out, dtype=np.float32)


# revision 17
# speedup vs baseline: 1.3379x; 1.3379x over previous
"""Trainium2 Bass kernel for nn_Cross_Attention_Block_3624952397825.

Mathematical structure exploited: the reference takes ``out[:, -1, :]`` --
the attention output of the LAST query token. That token comes from the
zero row appended by ``jnp.pad`` AFTER the conv stack, so its query vector
is exactly zero, its attention scores are exactly zero, and softmax over
exact zeros is exactly uniform (1/4096).  Hence

    bins[b]  = mean_k V[b, k, :]          = (mean_k lidar[b, k, :]) @ wv
    out[b]   = MLP3(leaky_relu chain)(bins[b])

The conv block, Q/K projections, and softmax are structurally dead code
for ANY input values.  The kernel therefore reduces lidar over its 4096
points on-device, applies wv and the 3-layer MLP, all on 8 NeuronCores
data-parallel over the batch (2 batches per core).

Implementation notes:
  * The point reduction is exact fp32: VectorE accumulates [128, 1024]
    tiles, folds to [128, 256], and a ones-column fp32 matmul folds the
    128 partitions (exact: the stationary operand is exactly 1.0).
  * The tiny MLP runs in fp16 (weights and activations; fp32 PSUM
    accumulation and fp32 biases) -- single-pass PE matmuls instead of
    the 2x half-speed fp32 mode.  Measured end-to-end error ~5e-4.
  * MLP weights ship in one packed fp16 [128, 3456] tensor split into 8
    column-chunk DMAs issued BEFORE the lidar stream; biases/ones in a
    tiny fp32 pack.  Lidar DMAs go batch 0 first so batch 0's MLP
    overlaps batch 1's stream.
"""

import numpy as np

B, NPTS, CH, DM = 16, 4096, 256, 1024
N_CORES = 8
BL = B // N_CORES            # batches per core
P = 128
TILE_F = 1024                # free dim of lidar tiles (4 pts x 256 ch)
N_TILES = NPTS * CH // (P * TILE_F)   # 8 tiles per batch

# fp16 weight pack layout (free dim)
OFF_WVS = 0                  # 2 k-chunks x 1024
OFF_WO1 = 2048               # 8 k-chunks x 128
OFF_WO2 = 3072               # 128
OFF_WO3 = 3200               # 256
W16_F = 3456
N_WCHUNK = 8
# fp32 pack columns
C_B1, C_B2, C_B3A, C_B3B, C_ONE = 0, 1, 2, 3, 4
W32_F = 8

_CACHE = {}


def _build_program():
    import concourse.bacc as bacc
    import concourse.mybir as mybir
    from concourse.tile import TileContext

    f32 = mybir.dt.float32
    f16 = mybir.dt.float16
    Alu = mybir.AluOpType
    Act = mybir.ActivationFunctionType

    nc = bacc.Bacc("TRN2")
    lidar = nc.dram_tensor("lidar", [BL, NPTS, CH], f32, kind="ExternalInput")
    wp16d = nc.dram_tensor("wp16", [P, W16_F], f16, kind="ExternalInput")
    wp32d = nc.dram_tensor("wp32", [P, W32_F], f32, kind="ExternalInput")
    out_rows = nc.dram_tensor("out_rows", [BL, CH], f32, kind="ExternalOutput")

    # [BL, 4096, 256] -> [(b t), 128, 1024]; per-partition rows are 4 KiB
    # contiguous in DRAM.
    lv = lidar[:, :, :].rearrange("b (t p q) c -> (b t) p (q c)", p=P, q=4)

    with TileContext(nc) as tc:
        with (
            tc.tile_pool(name="w", bufs=1) as wpool,
            tc.tile_pool(name="io", bufs=6) as iopool,
            tc.tile_pool(name="acc", bufs=1) as accpool,
            tc.tile_pool(name="small", bufs=1) as spool,
            tc.tile_pool(name="ps", bufs=2, space="PSUM") as pspool,
            tc.tile_pool(name="mm", bufs=3, space="PSUM") as mmpool,
        ):
            # weights first, split across all DMA queues so they land early
            wp16 = wpool.tile([P, W16_F], f16, tag="wp16")
            wc = W16_F // N_WCHUNK
            for j in range(N_WCHUNK):
                nc.sync.dma_start(out=wp16[:, j * wc:(j + 1) * wc],
                                  in_=wp16d[:, j * wc:(j + 1) * wc])
            wp32 = wpool.tile([P, W32_F], f32, tag="wp32")
            nc.sync.dma_start(out=wp32[:, :], in_=wp32d[:, :])
            ones_col = wp32[:, C_ONE:C_ONE + 1]

            for b in range(BL):
                # ---- exact fp32 point reduction ----
                acc = accpool.tile([P, TILE_F], f32, tag=f"acc{b}")
                nc.sync.dma_start(out=acc[:, :], in_=lv[b * N_TILES, :, :])
                for t in range(1, N_TILES):
                    tin = iopool.tile([P, TILE_F], f32, tag="tin")
                    nc.sync.dma_start(out=tin[:, :], in_=lv[b * N_TILES + t, :, :])
                    nc.vector.tensor_add(out=acc[:, :], in0=acc[:, :], in1=tin[:, :])
                # fold 1024 -> 256 (free layout: 4 points x 256 channels)
                nc.vector.tensor_add(out=acc[:, 0:512], in0=acc[:, 0:512],
                                     in1=acc[:, 512:1024])
                af = spool.tile([P, CH], f32, tag=f"af{b}")
                nc.vector.tensor_add(out=af[:, :], in0=acc[:, 0:256],
                                     in1=acc[:, 256:512])
                # fold 128 partitions with a ones-column matmul -> [1, 256]
                sred = pspool.tile([1, CH], f32, tag="sred")
                nc.tensor.matmul(sred[:, :], lhsT=ones_col, rhs=af[:, :],
                                 start=True, stop=True)
                s_sb = spool.tile([1, CH], f32, tag=f"ssb{b}")
                nc.scalar.copy(out=s_sb[:, :], in_=sred[:, :])
                # transpose row [1, 256] -> 2 x [128, 1]; convert to fp16
                # mean (x 1/4096) folded into the conversion scale
                mt = []
                for k in range(2):
                    mtp = mmpool.tile([P, 1], f32, tag="mm")
                    nc.tensor.matmul(mtp[:, :], lhsT=s_sb[0:1, k * P:(k + 1) * P],
                                     rhs=wp32[0:1, C_ONE:C_ONE + 1],
                                     start=True, stop=True)
                    mt16 = spool.tile([P, 1], f16, tag=f"mt{b}{k}")
                    nc.scalar.activation(mt16[:, :], mtp[:, :], Act.Copy,
                                         scale=float(1.0 / NPTS))
                    mt.append(mt16)

                # ---- fp16 MLP (fp32 PSUM accumulation, fp32 biases) ----
                vS = spool.tile([P, 8], f16, tag=f"vS{b}")
                for o in range(8):
                    vps = mmpool.tile([P, 1], f32, tag="mm")
                    for k in range(2):
                        base = OFF_WVS + k * 1024 + o * P
                        nc.tensor.matmul(vps[:, :], lhsT=wp16[:, base:base + P],
                                         rhs=mt[k][:, :], start=(k == 0), stop=(k == 1))
                    nc.scalar.copy(out=vS[:, o:o + 1], in_=vps[:, :])

                def leaky(zp, bias_col, tag):
                    z = spool.tile([P, 1], f16, tag=f"z{tag}")
                    nc.scalar.activation(z[:, :], zp[:, :], Act.Identity,
                                         bias=wp32[:, bias_col:bias_col + 1], scale=1.0)
                    h = spool.tile([P, 1], f16, tag=f"h{tag}")
                    nc.vector.scalar_tensor_tensor(out=h[:, :], in0=z[:, :], scalar=0.01,
                                                   in1=z[:, :], op0=Alu.mult, op1=Alu.max)
                    return h

                h1p = mmpool.tile([P, 1], f32, tag="mm")
                for k in range(8):
                    nc.tensor.matmul(h1p[:, :],
                                     lhsT=wp16[:, OFF_WO1 + k * P: OFF_WO1 + (k + 1) * P],
                                     rhs=vS[:, k:k + 1], start=(k == 0), stop=(k == 7))
                h1 = leaky(h1p, C_B1, f"1{b}")

                h2p = mmpool.tile([P, 1], f32, tag="mm")
                nc.tensor.matmul(h2p[:, :], lhsT=wp16[:, OFF_WO2:OFF_WO2 + P],
                                 rhs=h1[:, :], start=True, stop=True)
                h2 = leaky(h2p, C_B2, f"2{b}")

                for o in range(2):
                    ops = mmpool.tile([P, 1], f32, tag="mm")
                    nc.tensor.matmul(ops[:, :],
                                     lhsT=wp16[:, OFF_WO3 + o * P: OFF_WO3 + (o + 1) * P],
                                     rhs=h2[:, :], start=True, stop=True)
                    ofin = spool.tile([P, 1], f32, tag=f"ofin{b}{o}")
                    nc.scalar.activation(ofin[:, :], ops[:, :], Act.Identity,
                                         bias=wp32[:, C_B3A + o:C_B3A + o + 1], scale=1.0)
                    nc.sync.dma_start(out=out_rows[b:b + 1, o * P:(o + 1) * P],
                                      in_=ofin[:, :])

    nc.compile()
    return nc


def _pack_weights(inputs):
    wv = np.asarray(inputs["wv"], np.float32)
    wo1 = np.asarray(inputs["wo1"], np.float32)
    wo2 = np.asarray(inputs["wo2"], np.float32)
    wo3 = np.asarray(inputs["wo3"], np.float32)

    wp16 = np.zeros((P, W16_F), np.float16)
    wp16[:, OFF_WVS:OFF_WVS + 1024] = wv[0:128, :]
    wp16[:, OFF_WVS + 1024:OFF_WVS + 2048] = wv[128:256, :]
    for k in range(8):
        wp16[:, OFF_WO1 + k * P:OFF_WO1 + (k + 1) * P] = wo1[k * P:(k + 1) * P, :]
    wp16[:, OFF_WO2:OFF_WO2 + P] = wo2
    wp16[:, OFF_WO3:OFF_WO3 + CH] = wo3

    wp32 = np.zeros((P, W32_F), np.float32)
    wp32[:, C_B1] = np.asarray(inputs["b1"], np.float32)
    wp32[:, C_B2] = np.asarray(inputs["b2"], np.float32)
    wp32[:, C_B3A] = np.asarray(inputs["b3"], np.float32)[0:128]
    wp32[:, C_B3B] = np.asarray(inputs["b3"], np.float32)[128:256]
    wp32[:, C_ONE] = 1.0
    return wp16, wp32


def kernel(**inputs):
    from concourse.bass_utils import run_bass_kernel_spmd

    if "nc" not in _CACHE:
        _CACHE["nc"] = _build_program()
    nc = _CACHE["nc"]

    lidar = np.ascontiguousarray(np.asarray(inputs["lidar"], dtype=np.float32))
    wp16, wp32 = _pack_weights(inputs)

    in_maps = [
        {"lidar": lidar[i * BL:(i + 1) * BL], "wp16": wp16, "wp32": wp32}
        for i in range(N_CORES)
    ]
    res = run_bass_kernel_spmd(nc, in_maps, list(range(N_CORES)),
                               **_CACHE.get("run_kwargs", {}))
    _CACHE["last_results"] = res
    out = np.concatenate([res.results[i]["out_rows"] for i in range(N_CORES)], axis=0)
    return np.ascontiguousarray(out, dtype=np.float32)


# revision 18
# speedup vs baseline: 2.7526x; 2.0574x over previous
"""Trainium2 Bass kernel for nn_Cross_Attention_Block_3624952397825.

Mathematical structure exploited: the reference takes ``out[:, -1, :]`` --
the attention output of the LAST query token. That token comes from the
zero row appended by ``jnp.pad`` AFTER the conv stack, so its query vector
is exactly zero, its attention scores are exactly zero, and softmax over
exact zeros is exactly uniform (1/4096).  Hence

    bins[b] = mean_k V[b, k, :] = (mean_k lidar[b, k, :]) @ wv
    out[b]  = MLP3(leaky_relu chain)(bins[b])

The conv block, Q/K projections, and softmax are structurally dead code
for ANY input values.  Additionally there is no nonlinearity between wv
and wo1, so W1 = wv @ wo1 [256, 128] is constant-folded on the host.

Per core (2 batches): stream lidar as fp16 [128, 4096] tiles (8 KiB per
partition -> full single-queue DMA rate), reduce the 4096 points with
ones^T @ tile matmuls on TensorE (fp16 x fp16 products are exact for a
1.0 stationary; accumulation is fp32 in PSUM), then a tiny fp16 MLP
(fp32 biases, fp32 final add).  Weights ride the second HWDGE queue
(ScalarE) so the lidar FIFO is never interrupted; batch 0 streams first
so its MLP overlaps batch 1's DMA.  Measured model error ~6e-4.
"""

import numpy as np

B, NPTS, CH, DM = 16, 4096, 256, 1024
N_CORES = 8
BL = B // N_CORES            # batches per core
P = 128
TILE_F = 4096                # free dim of lidar tiles (16 pts x 256 ch)
N_TILES = NPTS * CH // (P * TILE_F)   # 2 tiles per batch

# fp16 weight pack layout (free dim)
OFF_W1 = 0                   # 2 k-chunks x 128   (W1 = wv @ wo1)
OFF_WO2 = 256                # 128
OFF_WO3 = 384                # 256  (stored [K=128, 256] for row-form output)
OFF_ONE16 = 640              # fp16 ones column
W16_F = 641
# fp32 pack columns
C_B1, C_B2 = 0, 1
W32_F = 4

_CACHE = {}


def _build_program():
    import concourse.bacc as bacc
    import concourse.mybir as mybir
    from concourse.tile import TileContext

    f32 = mybir.dt.float32
    f16 = mybir.dt.float16
    Alu = mybir.AluOpType
    Act = mybir.ActivationFunctionType

    nc = bacc.Bacc("TRN2")
    lidar = nc.dram_tensor("lidar16", [BL, NPTS, CH], f16, kind="ExternalInput")
    wp16d = nc.dram_tensor("wp16", [P, W16_F], f16, kind="ExternalInput")
    wp32d = nc.dram_tensor("wp32", [P, W32_F], f32, kind="ExternalInput")
    b3rowd = nc.dram_tensor("b3row", [1, CH], f32, kind="ExternalInput")
    out_rows = nc.dram_tensor("out_rows", [BL, CH], f32, kind="ExternalOutput")

    # [BL, 4096, 256] -> [(b t), 128, 4096]; 8 KiB contiguous per partition.
    lv = lidar[:, :, :].rearrange("b (t p q) c -> (b t) p (q c)", p=P, q=16)

    with TileContext(nc) as tc:
        with (
            tc.tile_pool(name="w", bufs=1) as wpool,
            tc.tile_pool(name="io", bufs=3) as iopool,
            tc.tile_pool(name="small", bufs=1) as spool,
            tc.tile_pool(name="ps", bufs=2, space="PSUM") as pspool,
            tc.tile_pool(name="orp", bufs=2, space="PSUM") as orpool,
            tc.tile_pool(name="mm", bufs=3, space="PSUM") as mmpool,
        ):
            # weights on the ScalarE HWDGE queue; lidar owns the SP queue
            wp16 = wpool.tile([P, W16_F], f16, tag="wp16")
            nc.scalar.dma_start(out=wp16[:, :], in_=wp16d[:, :])
            wp32 = wpool.tile([P, W32_F], f32, tag="wp32")
            nc.scalar.dma_start(out=wp32[:, :], in_=wp32d[:, :])
            b3row = wpool.tile([1, CH], f32, tag="b3row")
            nc.scalar.dma_start(out=b3row[:, :], in_=b3rowd[:, :])
            ones16 = wp16[:, OFF_ONE16:OFF_ONE16 + 1]

            for b in range(BL):
                # ---- point reduction: ones^T @ tile on TensorE ----
                # fp16 x 1.0 products are exact; fp32 PSUM accumulation.
                sred = pspool.tile([1, CH], f32, tag="sred")
                nmm = N_TILES * (TILE_F // CH)
                i = 0
                for t in range(N_TILES):
                    tin = iopool.tile([P, TILE_F], f16, tag="tin")
                    nc.sync.dma_start(out=tin[:, :], in_=lv[b * N_TILES + t, :, :])
                    for j in range(TILE_F // CH):
                        nc.tensor.matmul(sred[:, :], lhsT=ones16,
                                         rhs=tin[:, j * CH:(j + 1) * CH],
                                         start=(i == 0), stop=(i == nmm - 1))
                        i += 1
                # means in fp16 (mean scale folded into the conversion)
                s16 = spool.tile([1, CH], f16, tag=f"s16{b}")
                nc.scalar.activation(s16[:, :], sred[:, :], Act.Copy,
                                     scale=float(1.0 / NPTS))
                # transpose row [1, 256] -> 2 x [128, 1] via K=1 fp16 matmuls
                mt = []
                for k in range(2):
                    mtp = mmpool.tile([P, 1], f32, tag="mm")
                    nc.tensor.matmul(mtp[:, :], lhsT=s16[0:1, k * P:(k + 1) * P],
                                     rhs=ones16[0:1, 0:1], start=True, stop=True)
                    mt16 = spool.tile([P, 1], f16, tag=f"mt{b}{k}")
                    nc.scalar.copy(out=mt16[:, :], in_=mtp[:, :])
                    mt.append(mt16)

                def leaky(zp, bias_col, tag):
                    z = spool.tile([P, 1], f16, tag=f"z{tag}")
                    nc.scalar.activation(z[:, :], zp[:, :], Act.Identity,
                                         bias=wp32[:, bias_col:bias_col + 1], scale=1.0)
                    h = spool.tile([P, 1], f16, tag=f"h{tag}")
                    nc.vector.scalar_tensor_tensor(out=h[:, :], in0=z[:, :], scalar=0.01,
                                                   in1=z[:, :], op0=Alu.mult, op1=Alu.max)
                    return h

                # h1 = leaky(m @ W1 + b1), W1 pre-folded on host
                h1p = mmpool.tile([P, 1], f32, tag="mm")
                for k in range(2):
                    nc.tensor.matmul(h1p[:, :],
                                     lhsT=wp16[:, OFF_W1 + k * P: OFF_W1 + (k + 1) * P],
                                     rhs=mt[k][:, :], start=(k == 0), stop=(k == 1))
                h1 = leaky(h1p, C_B1, f"1{b}")

                h2p = mmpool.tile([P, 1], f32, tag="mm")
                nc.tensor.matmul(h2p[:, :], lhsT=wp16[:, OFF_WO2:OFF_WO2 + P],
                                 rhs=h1[:, :], start=True, stop=True)
                h2 = leaky(h2p, C_B2, f"2{b}")

                # final layer in row form: h2^T @ wo3 -> [1, 256]
                orp = orpool.tile([1, CH], f32, tag="orp")
                nc.tensor.matmul(orp[:, :], lhsT=h2[:, :],
                                 rhs=wp16[:, OFF_WO3:OFF_WO3 + CH],
                                 start=True, stop=True)
                orow = spool.tile([1, CH], f32, tag=f"orow{b}")
                nc.vector.tensor_add(out=orow[:, :], in0=orp[:, :], in1=b3row[:, :])
                nc.scalar.dma_start(out=out_rows[b:b + 1, :], in_=orow[:, :])

    nc.compile()
    return nc


def _pack_weights(inputs):
    wv = np.asarray(inputs["wv"], np.float64)
    wo1 = np.asarray(inputs["wo1"], np.float64)
    W1 = (wv @ wo1)                           # [256, 128], no nonlinearity between

    wp16 = np.zeros((P, W16_F), np.float16)
    wp16[:, OFF_W1:OFF_W1 + P] = W1[0:128, :]
    wp16[:, OFF_W1 + P:OFF_W1 + 2 * P] = W1[128:256, :]
    wp16[:, OFF_WO2:OFF_WO2 + P] = np.asarray(inputs["wo2"], np.float32)
    wp16[:, OFF_WO3:OFF_WO3 + CH] = np.asarray(inputs["wo3"], np.float32)
    wp16[:, OFF_ONE16] = 1.0

    wp32 = np.zeros((P, W32_F), np.float32)
    wp32[:, C_B1] = np.asarray(inputs["b1"], np.float32)
    wp32[:, C_B2] = np.asarray(inputs["b2"], np.float32)
    b3row = np.asarray(inputs["b3"], np.float32).reshape(1, CH)
    return wp16, wp32, b3row


def kernel(**inputs):
    from concourse.bass_utils import run_bass_kernel_spmd

    if "nc" not in _CACHE:
        _CACHE["nc"] = _build_program()
    nc = _CACHE["nc"]

    lidar16 = np.ascontiguousarray(
        np.asarray(inputs["lidar"], dtype=np.float32).astype(np.float16))
    wp16, wp32, b3row = _pack_weights(inputs)

    in_maps = [
        {"lidar16": lidar16[i * BL:(i + 1) * BL], "wp16": wp16,
         "wp32": wp32, "b3row": b3row}
        for i in range(N_CORES)
    ]
    res = run_bass_kernel_spmd(nc, in_maps, list(range(N_CORES)),
                               **_CACHE.get("run_kwargs", {}))
    _CACHE["last_results"] = res
    out = np.concatenate([res.results[i]["out_rows"] for i in range(N_CORES)], axis=0)
    return np.ascontiguousarray(out, dtype=np.float32)
